# revision 1
# baseline (speedup 1.0000x reference)
"""Bass/Tile Trainium2 kernel for causal MHA with RoPE (nn_MultiHeadAttention).

Problem (hardcoded): x [2,2048,1024] fp32, w_qkv [1024,3072], w_out [1024,1024],
16 heads, head_dim 64, rope theta 10000, causal softmax, out proj.

Sharding: 8 cores = 2 batches x 4 head-groups (4 heads each), Megatron-style
QKV column-split + out-proj row-split; host sums the 4 partial outputs per
batch.

Per-core design (all-transposed layout):
  qT/kT [d,s] f32r matmuls with RoPE fused at PSUM eviction (interleaved
  pair layout -> stream_shuffle pair swap), scoresT [sk,sq] with 2-head K=64
  row packing, causal mask added in PSUM via identity-matmul of a -1e9 mask,
  exp on ACT without max subtraction (scores are O(1)), row sums l via
  ones-matmuls packed 4-per-bank, 1/l broadcast via K=1 PE matmul,
  fp16 normalized probsT, PV with 2-head col packing, fp16 out-projection,
  all pipelined per 512-wide query superblock.
"""
import sys

sys.path.insert(0, "/opt/trn_rl_repo")
import numpy as np

N_EMBD = 1024
N_HEAD = 16
D = 64
B = 2
S = 2048
THETA = 10000.0

_PROG_CACHE = {}

PAIRSWAP = []
for _i in range(16):
    PAIRSWAP += [2 * _i + 1, 2 * _i]


def _subs(W):
    out = []
    off = 0
    while off < W:
        w = min(1024, W - off)
        out.append((off, w))
        off += w
    return out


def _build_program(n_reps=1, loop_n=0, stage="full"):
    import concourse.bass as bass
    from concourse import bacc
    import concourse.mybir as mybir
    from concourse.tile import TileContext
    from concourse import library_config

    f32 = mybir.dt.float32
    f32r = mybir.dt.float32r
    f16 = mybir.dt.float16
    EXP = mybir.ActivationFunctionType.Exp

    nc = bacc.Bacc("TRN2", target_bir_lowering=False, debug=False)

    xT = nc.declare_dram_parameter("xT", [1024, 2048], f32r, isOutput=False)
    wq = nc.declare_dram_parameter("wq", [128, 2048], f32r, isOutput=False)
    wk = nc.declare_dram_parameter("wk", [128, 2048], f32r, isOutput=False)
    wv = nc.declare_dram_parameter("wv", [128, 2048], f32r, isOutput=False)
    wo = nc.declare_dram_parameter("wo", [2, 128, 1024], f16, isOutput=False)
    cosT = nc.declare_dram_parameter("cosT", [128, 2048], f32, isOutput=False)
    sinT = nc.declare_dram_parameter("sinT", [128, 2048], f32, isOutput=False)
    msk = nc.declare_dram_parameter("msk", [128, 4, 512], f32r, isOutput=False)
    iden = nc.declare_dram_parameter("iden", [128, 128], f32r, isOutput=False)
    y = nc.declare_dram_parameter("y", [2048, 1024], f16, isOutput=True)

    MM = nc.tensor.matmul

    with nc.allow_low_precision(reason="fp16 probs/out path is intentional"), \
         TileContext(nc) as tc:
      from contextlib import ExitStack as _ES
      for _rep in range(n_reps):
        if _rep:
            tc.strict_bb_all_engine_barrier()
        with _ES() as _loopctx:
          if loop_n:
            _loopctx.enter_context(tc.For_i(0, loop_n, 1))
          with (
              tc.tile_pool(name="persist", bufs=1) as pp,
              tc.tile_pool(name="small", bufs=3) as sp,
          ):
              nc.gpsimd.load_library(library_config.standard)

              ones_col = pp.tile([128, 1], f16, tag="ones_col")
              ones_all = pp.tile([128, 128], f16, tag="ones_all")
              nc.gpsimd.memset(ones_col[:], 1.0)
              nc.gpsimd.memset(ones_all[:], 1.0)

              qsb = [pp.tile([128, 2048], f32r, tag=f"q{h}", name=f"q{h}") for h in range(2)]
              ksb = [pp.tile([128, 2048], f32r, tag=f"k{h}", name=f"k{h}") for h in range(2)]
              v_sb = pp.tile([128, 16, 256], f16, tag="v")
              outT = [pp.tile([128, 2048], f16, tag=f"o{h}", name=f"oT{h}") for h in range(2)]
              wo_sb = pp.tile([128, 2, 1024], f16, tag="wo")
              R_sb = [pp.tile([128, 2048], f16, tag=f"R{p}", name=f"R{p}") for p in range(2)]
              msk_sb = pp.tile([128, 4, 512], f32r, tag="msk")
              id_sb = pp.tile([128, 128], f32r, tag="iden")

              # ---------------- Phase 1: QKV + RoPE ----------------
              with (
                  tc.tile_pool(name="qkvp", bufs=1) as qp,
                  tc.tile_pool(name="rope", bufs=3) as rp,
                  tc.tile_pool(name="ps_qkv", bufs=1, space="PSUM") as ps1,
              ):
                  x_sb = qp.tile([128, 8, 2048], f32r, tag="x")
                  wq_sb = qp.tile([128, 2048], f32r, tag="wq")
                  wk_sb = qp.tile([128, 2048], f32r, tag="wk")
                  wv_sb = qp.tile([128, 2048], f32r, tag="wv")
                  cos_sb = qp.tile([128, 2048], f32, tag="cos")
                  sin_sb = qp.tile([128, 2048], f32, tag="sin")

                  nc.sync.dma_start(out=wq_sb[:], in_=wq[:])
                  nc.sync.dma_start(out=wk_sb[:], in_=wk[:])
                  for et in range(8):
                      nc.sync.dma_start(out=x_sb[:, et, :], in_=xT[et * 128:(et + 1) * 128, :])
                  nc.sync.dma_start(out=wv_sb[:], in_=wv[:])
                  nc.sync.dma_start(out=cos_sb[:], in_=cosT[:])
                  nc.sync.dma_start(out=sin_sb[:], in_=sinT[:])
                  nc.sync.dma_start(out=msk_sb[:], in_=msk[:])
                  nc.sync.dma_start(out=id_sb[:], in_=iden[:])
                  for hp in range(2):
                      nc.sync.dma_start(out=wo_sb[:, hp, :], in_=wo[hp])

                  if stage == "dma":
                      _skip_qkv = True
                  else:
                      _skip_qkv = False
                  # q and k in transposed layout, RoPE fused at eviction
                  # (hp outer so head-pair 0 is ready as early as possible)
                  for hp in range(2 if not _skip_qkv else 0):
                      for (wsb, dst, tg) in ((wq_sb, qsb, "q"), (wk_sb, ksb, "k")):
                          for c4 in range(4):
                              sl = slice(c4 * 512, (c4 + 1) * 512)
                              pq = ps1.tile([128, 512], f32, tag=f"ps_{tg}", bufs=3)
                              for et in range(8):
                                  MM(pq[:], wsb[:, (et * 2 + hp) * 128:(et * 2 + hp + 1) * 128],
                                     x_sb[:, et, sl], start=(et == 0), stop=(et == 7))
                              t1 = rp.tile([128, 512], f32, tag="t1")
                              nc.vector.tensor_mul(t1[:], pq[:], cos_sb[:, sl])
                              rot = rp.tile([128, 512], f32, tag="rot")
                              nc.vector.stream_shuffle(rot[:], pq[:], PAIRSWAP)
                              t2 = rp.tile([128, 512], f32, tag="t2")
                              nc.gpsimd.tensor_mul(t2[:], rot[:], sin_sb[:, sl])
                              nc.gpsimd.tensor_add(dst[hp][:, sl], t1[:], t2[:])

                  # v in natural layout [s, d_local]
                  for st in range(16 if not _skip_qkv else 0):
                      pv = ps1.tile([128, 256], f32, tag="ps_v", bufs=2)
                      for et in range(8):
                          MM(pv[:], x_sb[:, et, st * 128:(st + 1) * 128],
                             wv_sb[:, et * 256:(et + 1) * 256], start=(et == 0), stop=(et == 7))
                      nc.scalar.copy(v_sb[:, st, :], pv[:])

              # -------- Phase 2+3: attention + outproj, pipelined per superblock ----
              if stage in ("dma", "qkv"):
                  nc.finalize  # no-op reference
              with (
                  tc.tile_pool(name="probs", bufs=1) as probp,
                  tc.tile_pool(name="ps_att", bufs=1, space="PSUM") as ps2,
              ):
                  for hp in range(2 if stage not in ("dma", "qkv") else 0):
                      probs = {}
                      l_ps = [ps2.tile([128, 512], f32, tag=f"l{p}", name=f"l{p}")
                              for p in range(2)]
                      for j in range(16):
                          sq0 = (j // 4) * 512
                          W = 2048 - sq0
                          jj = slice(j * 128, (j + 1) * 128)
                          for par in range(2):
                              probs[(j, par)] = probp.tile(
                                  [128, W], f16, tag=f"p{j}_{par}", name=f"pr{j}_{par}", bufs=2 if j <= 5 else 1)
                          for (off, w) in _subs(W):
                              psT = [ps2.tile([128, 1024], f32, tag=f"sT{p}", name=f"sT{p}")
                                     for p in range(2)]
                              for half in range(w // 512):
                                  qsl = slice(sq0 + off + half * 512,
                                              sq0 + off + (half + 1) * 512)
                                  diag = (off == 0 and half == 0)
                                  m = j % 4
                                  for par in range(2):
                                      pb = slice(64 * par, 64 * par + 64)
                                      osl = slice(half * 512, (half + 1) * 512)
                                      if diag:
                                          # causal mask add into PSUM: Id.T @ mask_j
                                          # (mask only nonzero in first (m+1)*128
                                          # cols; scores fully masked below m*128)
                                          wm = max(256, (m + 1) * 128)
                                          MM(psT[par][:, 0:wm], id_sb[:],
                                             msk_sb[:, m, 0:wm],
                                             start=True, stop=False,
                                             skip_group_check=True)
                                          ws = max(256, 512 - m * 128)
                                          MM(psT[par][:, 512 - ws:512],
                                             ksb[hp][pb, jj],
                                             qsb[hp][pb, sq0 + 512 - ws:sq0 + 512],
                                             start=False, stop=True,
                                             skip_group_check=True)
                                      else:
                                          MM(psT[par][:, osl],
                                             ksb[hp][pb, jj], qsb[hp][pb, qsl],
                                             start=True, stop=True,
                                             skip_group_check=True)
                              for par in range(2):
                                  nc.scalar.activation(
                                      probs[(j, par)][:, off:off + w], psT[par][:, :w], EXP)
                          # l accumulation (row sums via ones matmuls, 4 rows per bank)
                          for par in range(2):
                              for I in range(j // 4, 4):
                                  psl = slice(I * 512 - sq0, I * 512 - sq0 + 512)
                                  MM(l_ps[par][32 * I:32 * I + 1, :], ones_col[:],
                                     probs[(j, par)][:, psl],
                                     start=(j == 0), stop=(j == 4 * I + 3),
                                     skip_group_check=True,
                                     tile_position=(0, 32 * I))

                          if j % 4 == 3:
                              I = j // 4
                              Rsl = slice(I * 512, (I + 1) * 512)
                              # superblock I complete: 1/l, broadcast, normalize, PV
                              for par in range(2):
                                  rI = sp.tile([1, 512], f16, tag="r")
                                  nc.vector.reciprocal(
                                      rI[:], l_ps[par][32 * I:32 * I + 1, :])
                                  Rps = ps2.tile([128, 512], f32, tag="misc_ps", name="Rps")
                                  MM(Rps[:], ones_all[0:1, :], rI[:])
                                  if par == 0:
                                      nc.scalar.copy(R_sb[par][:, Rsl], Rps[:])
                                  else:
                                      nc.vector.tensor_copy(R_sb[par][:, Rsl], Rps[:])
                              for par in range(2):
                                  for j2 in range(4 * I + 4):
                                      sq2 = (j2 // 4) * 512
                                      psl = slice(I * 512 - sq2, I * 512 - sq2 + 512)
                                      eng = nc.gpsimd if (j2 % 4 == 1) else nc.vector
                                      eng.tensor_mul(probs[(j2, par)][:, psl],
                                                     probs[(j2, par)][:, psl],
                                                     R_sb[par][:, Rsl])
                              pvp = ps2.tile([128, 512], f32, tag="pv_ps")
                              for j2 in range(4 * I + 4):
                                  sq2 = (j2 // 4) * 512
                                  psl = slice(I * 512 - sq2, I * 512 - sq2 + 512)
                                  for par in range(2):
                                      MM(pvp[64 * par:64 * par + 64, :],
                                         v_sb[:, j2,
                                              (2 * hp + par) * 64:(2 * hp + par + 1) * 64],
                                         probs[(j2, par)][:, psl],
                                         start=(j2 == 0), stop=(j2 == 4 * I + 3),
                                         skip_group_check=True)
                              nc.vector.tensor_copy(outT[hp][:, Rsl], pvp[:])

                              if hp == 1 and stage == "full":
                                  # out projection for this superblock's 4 row tiles
                                  for t in range(4 * I, 4 * I + 4):
                                      for n in range(2):
                                          py = ps2.tile([128, 512], f32, tag="misc_ps",
                                                        name="py")
                                          for h2 in range(2):
                                              MM(py[:], outT[h2][:, t * 128:(t + 1) * 128],
                                                 wo_sb[:, h2, n * 512:(n + 1) * 512],
                                                 start=(h2 == 0), stop=(h2 == 1))
                                          ysb = sp.tile([128, 512], f16, tag="y_sb")
                                          nc.vector.tensor_copy(ysb[:], py[:])
                                          nc.sync.dma_start(
                                              out=y[t * 128:(t + 1) * 128,
                                                    n * 512:(n + 1) * 512],
                                              in_=ysb[:])

    nc.finalize()
    return nc


def _host_inputs(x, w_qkv, w_out, core):
    b = core // 4
    g = core % 4
    heads = [4 * g + i for i in range(4)]

    perm = np.empty(64, np.int64)
    for i in range(32):
        perm[2 * i] = i
        perm[2 * i + 1] = i + 32

    xT = np.ascontiguousarray(x[b].T).astype(np.float32)

    def build_wqk(Wblk, scale):
        out = np.empty((128, 2048), np.float32)
        for et in range(8):
            for hp in range(2):
                cols = np.empty((128, 128), np.float32)
                for par in range(2):
                    h = heads[2 * hp + par]
                    cols[:, par * 64:(par + 1) * 64] = (
                        Wblk[et * 128:(et + 1) * 128, h * 64 + perm])
                out[:, (et * 2 + hp) * 128:(et * 2 + hp + 1) * 128] = cols
        return (out * scale).astype(np.float32)

    wq = build_wqk(w_qkv[:, 0:1024], 0.125)
    wk = build_wqk(w_qkv[:, 1024:2048], 1.0)

    wv = np.empty((128, 2048), np.float32)
    for et in range(8):
        for hl in range(4):
            wv[:, et * 256 + hl * 64: et * 256 + (hl + 1) * 64] = (
                w_qkv[et * 128:(et + 1) * 128,
                      2048 + heads[hl] * 64: 2048 + (heads[hl] + 1) * 64])

    wo = np.empty((2, 128, 1024), np.float16)
    for hp in range(2):
        for par in range(2):
            h = heads[2 * hp + par]
            wo[hp, par * 64:(par + 1) * 64, :] = (
                w_out[h * 64:(h + 1) * 64, :].astype(np.float16))

    invf = 1.0 / (THETA ** (np.arange(0, D, 2, dtype=np.float64) / D))
    ang = np.arange(S, dtype=np.float64)[:, None] * invf[None, :]
    cosv = np.cos(ang).astype(np.float32)
    sinv = np.sin(ang).astype(np.float32)
    cosT = np.empty((128, 2048), np.float32)
    sinT = np.empty((128, 2048), np.float32)
    for r in range(64):
        i = r // 2
        sgn = -1.0 if (r % 2 == 0) else 1.0
        cosT[r, :] = cosv[:, i]
        cosT[r + 64, :] = cosv[:, i]
        sinT[r, :] = sgn * sinv[:, i]
        sinT[r + 64, :] = sgn * sinv[:, i]

    # causal masks [128, 4, 512]: msk[p, m, c] = 0 if c >= m*128+p else -1e9
    p_ = np.arange(128)[:, None, None]
    m_ = np.arange(4)[None, :, None]
    c_ = np.arange(512)[None, None, :]
    mskh = np.where(c_ >= m_ * 128 + p_, 0.0, -1e9).astype(np.float32)
    iden = np.eye(128, dtype=np.float32)

    return {
        "xT": xT, "wq": wq, "wk": wk, "wv": wv, "wo": wo,
        "cosT": cosT, "sinT": sinT, "msk": mskh, "iden": iden,
    }


def kernel(x, w_qkv, w_out, trace=False):
    from concourse.bass_utils import run_bass_kernel_spmd

    x = np.asarray(x, np.float32)
    w_qkv = np.asarray(w_qkv, np.float32)
    w_out = np.asarray(w_out, np.float32)

    if "nc" not in _PROG_CACHE:
        _PROG_CACHE["nc"] = _build_program()
    nc = _PROG_CACHE["nc"]

    in_maps = [_host_inputs(x, w_qkv, w_out, c) for c in range(8)]
    res = run_bass_kernel_spmd(nc, in_maps, list(range(8)), trace=trace)
    _PROG_CACHE["last_result"] = res

    y0 = sum(res.results[c]["y"].astype(np.float64) for c in range(4))
    y1 = sum(res.results[c]["y"].astype(np.float64) for c in range(4, 8))
    return np.stack([y0, y1]).astype(np.float32)



# revision 3
# speedup vs baseline: 1.0527x; 1.0527x over previous
"""Bass/Tile Trainium2 kernel v2 for causal MHA with RoPE (nn_MultiHeadAttention).

Problem (hardcoded): x [2,2048,1024] fp32, w_qkv [1024,3072], w_out [1024,1024],
16 heads, head_dim 64, rope theta 10000, causal softmax, out proj.

Sharding: 8 cores = 2 batches x 4 head-groups (4 heads each), Megatron-style
QKV column-split + out-proj row-split; host sums the 4 partial outputs per
batch.

v2 design vs baseline:
  - fp16 everywhere on the matmul paths (x, w, q, k, v, probs, out) --
    1 cyc/row at any N, half the DMA/SBUF footprint.
  - query-superblock-outer loop: for I (512 queries): for hp: stream key
    tiles j, with scores -> exp -> {l-acc, PV-acc} fully pipelined per j;
    probs tiles are consumed immediately (small rotating pool).
  - post-PV normalization: PV runs on unnormalized exp(scores-4) probs;
    1/l is broadcast via a K=2 selector matmul and applied once at PSUM
    eviction of pvp (removes 80 probs-normalize muls + R broadcast copies).
  - exp batched per j across both packed heads ([128,2,512] PSUM pair),
    with exp bias -4 as fp16-overflow guard.
  - fine-grained causality on the 4 diagonal key tiles per superblock
    (column range [128m, 512) only) - saves ~15% of scores/exp/l/PV.
  - diag mask: score MM first (start=True, clears bank), then a single
    [128,128] triangle mask MM added on top (sim-compatible order).
"""
import sys

sys.path.insert(0, "/opt/trn_rl_repo")
import numpy as np

N_EMBD = 1024
N_HEAD = 16
D = 64
B = 2
S = 2048
THETA = 10000.0
EXPBIAS = -4.0

_PROG_CACHE = {}

PAIRSWAP = []
for _i in range(16):
    PAIRSWAP += [2 * _i + 1, 2 * _i]


def _build_program(n_reps=1, loop_n=0, timing=False, ablate=()):
    import concourse.bass as bass
    from concourse import bacc
    import concourse.mybir as mybir
    from concourse.tile import TileContext
    from concourse import library_config
    from contextlib import ExitStack as _ES

    f32 = mybir.dt.float32
    f16 = mybir.dt.float16
    EXP = mybir.ActivationFunctionType.Exp

    nc = bacc.Bacc("TRN2", target_bir_lowering=False, debug=False)

    if timing:
        # timing-only build: no big external I/O (kills per-call transfer cost)
        din = lambda name, shape, dt_: nc.dram_tensor(name, shape, dt_, kind="Internal")
    else:
        din = lambda name, shape, dt_: nc.declare_dram_parameter(name, shape, dt_, isOutput=False)
    xT = din("xT", [8, 128, 2048], f16)
    wq = din("wq", [128, 2048], f16)
    wk = din("wk", [128, 2048], f16)
    wv = din("wv", [128, 2048], f16)
    wo = din("wo", [2, 128, 1024], f16)
    cosT = din("cosT", [128, 2048], f32)
    sinT = din("sinT", [128, 2048], f32)
    mskt = din("mskt", [128, 128], f16)
    iden = din("iden", [128, 128], f16)
    lsel = din("lsel", [2, 1, 128], f16)
    if timing:
        y = nc.dram_tensor("y", [2048, 1024], f16, kind="Internal")
        ytick = nc.declare_dram_parameter("ytick", [128, 128], f16, isOutput=True)
    else:
        y = nc.declare_dram_parameter("y", [2048, 1024], f16, isOutput=True)

    MM = nc.tensor.matmul

    with nc.allow_low_precision(reason="fp16 matmul paths are intentional"), \
         TileContext(nc) as tc:
      for _rep in range(n_reps):
        if _rep:
            tc.strict_bb_all_engine_barrier()
        with _ES() as _loopctx:
          if loop_n:
            _loopctx.enter_context(tc.For_i(0, loop_n, 1))
          with (
            tc.tile_pool(name="persist", bufs=1) as pp,
            tc.tile_pool(name="small", bufs=3) as sp,
          ):
            if timing:
                tk = sp.tile([128, 128], f16, tag="tick")

            nc.gpsimd.load_library(library_config.standard)

            ones_col = pp.tile([128, 1], f16, tag="ones_col")
            nc.gpsimd.memset(ones_col[:], 1.0)
            bias_sb = pp.tile([128, 1], f32, tag="bias")
            nc.gpsimd.memset(bias_sb[:], EXPBIAS)

            qsb = [pp.tile([128, 2048], f16, tag=f"q{h}", name=f"q{h}") for h in range(2)]
            ksb = [pp.tile([128, 2048], f16, tag=f"k{h}", name=f"k{h}") for h in range(2)]
            v_sb = pp.tile([128, 16, 256], f16, tag="v")
            outT = [pp.tile([128, 2048], f16, tag=f"o{h}", name=f"oT{h}") for h in range(2)]
            wo_sb = pp.tile([128, 2, 1024], f16, tag="wo")
            tri_sb = pp.tile([128, 2, 128], f16, tag="tri")
            lsel_sb = [pp.tile([1, 128], f16, tag=f"lsel{i}", name=f"lsel{i}")
                       for i in range(2)]

            # ---------------- Phase 1: QKV + RoPE ----------------
            with (
                tc.tile_pool(name="qkvp", bufs=1) as qp,
                tc.tile_pool(name="rope", bufs=3) as rp,
                tc.tile_pool(name="ps_qkv", bufs=1, space="PSUM") as ps1,
            ):
                x_sb = qp.tile([128, 8, 2048], f16, tag="x")
                wq_sb = qp.tile([128, 2048], f16, tag="wq")
                wk_sb = qp.tile([128, 2048], f16, tag="wk")
                wv_sb = qp.tile([128, 2048], f16, tag="wv")
                cos_sb = qp.tile([128, 2048], f32, tag="cos")
                sin_sb = qp.tile([128, 2048], f32, tag="sin")

                # spread input DMAs across engine queues so they parallelize:
                # x blocks on SP, weights on ACT, cos/sin on DVE, rest on Pool
                nc.scalar.dma_start(out=wq_sb[:], in_=wq[:])
                nc.scalar.dma_start(out=wk_sb[:], in_=wk[:])
                for c4 in range(4):
                    sl = slice(c4 * 512, (c4 + 1) * 512)
                    for et in range(8):
                        nc.sync.dma_start(out=x_sb[:, et, sl], in_=xT[et][:, sl])
                for ch in range(2):
                    csl = slice(ch * 1024, (ch + 1) * 1024)
                    nc.gpsimd.dma_start(out=cos_sb[:, csl], in_=cosT[:, csl])
                    nc.gpsimd.dma_start(out=sin_sb[:, csl], in_=sinT[:, csl])
                nc.gpsimd.dma_start(out=wv_sb[:], in_=wv[:])
                for i in range(2):
                    nc.gpsimd.dma_start(out=tri_sb[:, i, :], in_=mskt[:])
                for i in range(2):
                    nc.gpsimd.dma_start(out=lsel_sb[i][:], in_=lsel[i])
                for hp2 in range(2):
                    nc.gpsimd.dma_start(out=wo_sb[:, hp2, :], in_=wo[hp2])

                # preload the exp table set off the critical path
                warm = sp.tile([128, 1], f32, tag="warm")
                nc.scalar.activation(warm[:], ones_col[:], EXP)

                def qk_rope(hp):
                    for (wsb, dst, tg) in ((wq_sb, qsb, "q"), (wk_sb, ksb, "k")):
                        for c2 in range(2):
                            # [128, 1024] psum pair; rope eviction batched
                            pq = ps1.tile([128, 1024], f32, tag="ps_qk", bufs=3)
                            for half in range(2):
                                sl = slice(c2 * 1024 + half * 512,
                                           c2 * 1024 + (half + 1) * 512)
                                for et in range(8):
                                    MM(pq[:, half * 512:(half + 1) * 512],
                                       wsb[:, (et * 2 + hp) * 128:(et * 2 + hp + 1) * 128],
                                       x_sb[:, et, sl], start=(et == 0), stop=(et == 7))
                            csl = slice(c2 * 1024, (c2 + 1) * 1024)
                            t1 = rp.tile([128, 1024], f32, tag="t1")
                            nc.vector.tensor_mul(t1[:], pq[:], cos_sb[:, csl])
                            rot = rp.tile([128, 1024], f32, tag="rot")
                            nc.vector.stream_shuffle(rot[:], pq[:], PAIRSWAP)
                            t2 = rp.tile([128, 1024], f32, tag="t2")
                            nc.gpsimd.tensor_mul(t2[:], rot[:], sin_sb[:, csl])
                            nc.gpsimd.tensor_add(dst[hp][:, csl], t1[:], t2[:])

                def v_proj(st0, st1):
                    for st in range(st0, st1):
                        pv = ps1.tile([128, 256], f32, tag="ps_v", bufs=2)
                        for et in range(8):
                            MM(pv[:], x_sb[:, et, st * 128:(st + 1) * 128],
                               wv_sb[:, et * 256:(et + 1) * 256],
                               start=(et == 0), stop=(et == 7))
                        nc.vector.tensor_copy(v_sb[:, st, :], pv[:])

                qk_rope(0)
                qk_rope(1)
                v_proj(0, 16)

            # -------- Phase 2+3: attention + outproj, per query superblock --
            with (
                tc.tile_pool(name="probs", bufs=1) as probp,
                tc.tile_pool(name="ps_att", bufs=1, space="PSUM") as ps2,
            ):
                def emit_finalize(I, hp, l_ps, pvp):
                    # 1/l broadcast via two accumulating K=1 selector MMs
                    # (partition-strided APs are rejected by walrus)
                    Isl = slice(I * 512, (I + 1) * 512)
                    rIa = sp.tile([1, 512], f16, tag="rIa")
                    nc.vector.reciprocal(rIa[:], l_ps[0:1, :])
                    rIb = sp.tile([1, 512], f16, tag="rIb")
                    nc.vector.reciprocal(rIb[:], l_ps[64:65, :])
                    Rps = ps2.tile([128, 512], f32, tag="psT", name="Rps",
                                   bufs=3)
                    MM(Rps[:], lsel_sb[0][:], rIa[:], start=True, stop=False)
                    MM(Rps[:], lsel_sb[1][:], rIb[:], start=False, stop=True)
                    Rsb = sp.tile([128, 512], f16, tag="Rsb")
                    nc.vector.tensor_copy(Rsb[:], Rps[:])
                    nc.vector.tensor_mul(outT[hp][:, Isl], pvp[:], Rsb[:])
                    if hp == 1:
                        for t in range(4 * I, 4 * I + 4):
                            for n in range(2):
                                py = ps2.tile([128, 512], f32, tag="psT",
                                              name="py", bufs=3)
                                for h2 in range(2):
                                    MM(py[:],
                                       outT[h2][:, t * 128:(t + 1) * 128],
                                       wo_sb[:, h2, n * 512:(n + 1) * 512],
                                       start=(h2 == 0), stop=(h2 == 1))
                                ysb = sp.tile([128, 512], f16, tag="y_sb")
                                nc.vector.tensor_copy(ysb[:], py[:])
                                nc.sync.dma_start(
                                    out=y[t * 128:(t + 1) * 128,
                                          n * 512:(n + 1) * 512],
                                    in_=ysb[:])

                pending = None
                for I in range(4):
                    for hp in range(2):
                        nj = 4 * I + 4
                        l_ps = ps2.tile([128, 512], f32, tag="l", name="l_ps",
                                        bufs=1)
                        pvp = ps2.tile([128, 512], f32, tag="pv", name="pvp",
                                       bufs=1)
                        for j in range(nj):
                            diag = j >= 4 * I
                            m = (j - 4 * I) if diag else 0
                            s = m * 128
                            jj = slice(j * 128, (j + 1) * 128)
                            qsl = slice(I * 512 + s, (I + 1) * 512)
                            psT = ps2.tile([128, 2, 512], f32, tag="psT",
                                           name="psT", bufs=3)
                            for par in range(2):
                                pb = slice(64 * par, 64 * par + 64)
                                MM(psT[:, par, s:], ksb[hp][pb, jj],
                                   qsb[hp][pb, qsl],
                                   start=True, stop=True,
                                   skip_group_check=True)
                            probs = probp.tile([128, 2, 512], f16, tag="probs",
                                               name="probs", bufs=6)
                            if "exp" not in ablate:
                                nc.scalar.activation(probs[:, :, s:],
                                                     psT[:, :, s:],
                                                     EXP, bias=bias_sb[:])
                            else:
                                nc.scalar.activation(probs[:, :, s:s + 64],
                                                     psT[:, :, s:s + 64],
                                                     EXP, bias=bias_sb[:])
                            if diag and "mask" not in ablate:
                                nc.vector.tensor_mul(probs[:, :, s:s + 128],
                                                     probs[:, :, s:s + 128],
                                                     tri_sb[:])
                            if j == 1 and pending is not None:
                                emit_finalize(*pending)
                                pending = None
                            if "l" not in ablate:
                              for par in range(2):
                                MM(l_ps[64 * par:64 * par + 1, s:], ones_col[:],
                                   probs[:, par, s:],
                                   start=(j == 0), stop=(j == nj - 1),
                                   skip_group_check=True,
                                   tile_position=(0, 64 * par))
                            else:
                                MM(l_ps[0:1, s:s + 64], ones_col[:],
                                   probs[:, 0, s:s + 64],
                                   start=(j == 0), stop=(j == nj - 1),
                                   skip_group_check=True)
                            if "pv" not in ablate:
                              for par in range(2):
                                MM(pvp[64 * par:64 * par + 64, s:],
                                   v_sb[:, j, (2 * hp + par) * 64:(2 * hp + par + 1) * 64],
                                   probs[:, par, s:],
                                   start=(j == 0), stop=(j == nj - 1),
                                   skip_group_check=True)
                            else:
                                MM(pvp[0:64, s:s + 64],
                                   v_sb[:, j, (2 * hp) * 64:(2 * hp + 1) * 64],
                                   probs[:, 0, s:s + 64],
                                   start=(j == 0), stop=(j == nj - 1),
                                   skip_group_check=True)
                        pending = (I, hp, l_ps, pvp)
                emit_finalize(*pending)

            if timing:
                nc.sync.dma_start(out=tk[:], in_=y[1920:2048, 896:1024])
                nc.sync.dma_start(out=ytick[:], in_=tk[:])

    nc.finalize()
    return nc


def _host_inputs(x, w_qkv, w_out, core):
    b = core // 4
    g = core % 4
    heads = [4 * g + i for i in range(4)]

    perm = np.empty(64, np.int64)
    for i in range(32):
        perm[2 * i] = i
        perm[2 * i + 1] = i + 32

    xTh = np.ascontiguousarray(x[b].T).astype(np.float16)
    xT = np.empty((8, 128, 2048), np.float16)
    for et in range(8):
        xT[et] = xTh[et * 128:(et + 1) * 128]

    def build_wqk(Wblk, scale):
        out = np.empty((128, 2048), np.float32)
        for et in range(8):
            for hp in range(2):
                cols = np.empty((128, 128), np.float32)
                for par in range(2):
                    h = heads[2 * hp + par]
                    cols[:, par * 64:(par + 1) * 64] = (
                        Wblk[et * 128:(et + 1) * 128, h * 64 + perm])
                out[:, (et * 2 + hp) * 128:(et * 2 + hp + 1) * 128] = cols
        return (out * scale).astype(np.float16)

    wq = build_wqk(w_qkv[:, 0:1024], 0.125)
    wk = build_wqk(w_qkv[:, 1024:2048], 1.0)

    wv = np.empty((128, 2048), np.float16)
    for et in range(8):
        for hl in range(4):
            wv[:, et * 256 + hl * 64: et * 256 + (hl + 1) * 64] = (
                w_qkv[et * 128:(et + 1) * 128,
                      2048 + heads[hl] * 64: 2048 + (heads[hl] + 1) * 64])

    wo = np.empty((2, 128, 1024), np.float16)
    for hp in range(2):
        for par in range(2):
            h = heads[2 * hp + par]
            wo[hp, par * 64:(par + 1) * 64, :] = (
                w_out[h * 64:(h + 1) * 64, :].astype(np.float16))

    invf = 1.0 / (THETA ** (np.arange(0, D, 2, dtype=np.float64) / D))
    ang = np.arange(S, dtype=np.float64)[:, None] * invf[None, :]
    cosv = np.cos(ang).astype(np.float32)
    sinv = np.sin(ang).astype(np.float32)
    cosT = np.empty((128, 2048), np.float32)
    sinT = np.empty((128, 2048), np.float32)
    for r in range(64):
        i = r // 2
        sgn = -1.0 if (r % 2 == 0) else 1.0
        cosT[r, :] = cosv[:, i]
        cosT[r + 64, :] = cosv[:, i]
        sinT[r, :] = sgn * sinv[:, i]
        sinT[r + 64, :] = sgn * sinv[:, i]

    # [128,128] lower-triangle mask: mskt[p, c] = 0 if c >= p else -30000
    p_ = np.arange(128)[:, None]
    c_ = np.arange(128)[None, :]
    mskt = np.where(c_ >= p_, 1.0, 0.0).astype(np.float16)
    iden = np.eye(128, dtype=np.float16)
    lsel = np.zeros((2, 1, 128), np.float16)
    lsel[0, 0, 0:64] = 1.0
    lsel[1, 0, 64:128] = 1.0

    return {
        "xT": xT, "wq": wq, "wk": wk, "wv": wv, "wo": wo,
        "cosT": cosT, "sinT": sinT, "mskt": mskt, "iden": iden, "lsel": lsel,
    }


def kernel(x, w_qkv, w_out, trace=False):
    from concourse.bass_utils import run_bass_kernel_spmd

    x = np.asarray(x, np.float32)
    w_qkv = np.asarray(w_qkv, np.float32)
    w_out = np.asarray(w_out, np.float32)

    if "nc" not in _PROG_CACHE:
        _PROG_CACHE["nc"] = _build_program()
    nc = _PROG_CACHE["nc"]

    in_maps = [_host_inputs(x, w_qkv, w_out, c) for c in range(8)]
    res = run_bass_kernel_spmd(nc, in_maps, list(range(8)), trace=trace)
    _PROG_CACHE["last_result"] = res

    y0 = sum(res.results[c]["y"].astype(np.float64) for c in range(4))
    y1 = sum(res.results[c]["y"].astype(np.float64) for c in range(4, 8))
    return np.stack([y0, y1]).astype(np.float32)


# revision 5
# speedup vs baseline: 1.3753x; 1.3065x over previous
"""Bass/Tile Trainium2 kernel v2 for causal MHA with RoPE (nn_MultiHeadAttention).

Problem (hardcoded): x [2,2048,1024] fp32, w_qkv [1024,3072], w_out [1024,1024],
16 heads, head_dim 64, rope theta 10000, causal softmax, out proj.

Sharding: 8 cores = 2 batches x 4 head-groups (4 heads each), Megatron-style
QKV column-split + out-proj row-split; host sums the 4 partial outputs per
batch.

v2 design vs baseline:
  - fp16 everywhere on the matmul paths (x, w, q, k, v, probs, out) --
    1 cyc/row at any N, half the DMA/SBUF footprint.
  - query-superblock-outer loop: for I (512 queries): for hp: stream key
    tiles j, with scores -> exp -> {l-acc, PV-acc} fully pipelined per j;
    probs tiles are consumed immediately (small rotating pool).
  - post-PV normalization: PV runs on unnormalized exp(scores-4) probs;
    1/l is broadcast via a K=2 selector matmul and applied once at PSUM
    eviction of pvp (removes 80 probs-normalize muls + R broadcast copies).
  - exp batched per j across both packed heads ([128,2,512] PSUM pair),
    with exp bias -4 as fp16-overflow guard.
  - fine-grained causality on the 4 diagonal key tiles per superblock
    (column range [128m, 512) only) - saves ~15% of scores/exp/l/PV.
  - diag mask applied post-exp as a DVE 0/1 triangle multiply (cheaper
    than identity-matmul mask adds, which cost ~556ns each on HW).
  - v-projection chains interleaved into the attention phase (borrowing
    scores-PSUM slots), so their PE work fills the bubbles of the
    exp-paced softmax pipeline.
"""
import sys

sys.path.insert(0, "/opt/trn_rl_repo")
import numpy as np

N_EMBD = 1024
N_HEAD = 16
D = 64
B = 2
S = 2048
THETA = 10000.0
EXPBIAS = -4.0

_PROG_CACHE = {}

PAIRSWAP = []
for _i in range(16):
    PAIRSWAP += [2 * _i + 1, 2 * _i]


def _build_program(n_reps=1, loop_n=0, timing=False, ablate=()):
    import concourse.bass as bass
    from concourse import bacc
    import concourse.mybir as mybir
    from concourse.tile import TileContext
    from concourse import library_config
    from contextlib import ExitStack as _ES

    f32 = mybir.dt.float32
    f16 = mybir.dt.float16
    EXP = mybir.ActivationFunctionType.Exp

    nc = bacc.Bacc("TRN2", target_bir_lowering=False, debug=False)

    if timing:
        # timing-only build: no big external I/O (kills per-call transfer cost)
        din = lambda name, shape, dt_: nc.dram_tensor(name, shape, dt_, kind="Internal")
    else:
        din = lambda name, shape, dt_: nc.declare_dram_parameter(name, shape, dt_, isOutput=False)
    xT = din("xT", [8, 128, 2048], f16)
    wq = din("wq", [128, 2048], f16)
    wk = din("wk", [128, 2048], f16)
    wv = din("wv", [128, 2048], f16)
    wo = din("wo", [2, 128, 1024], f16)
    cosT = din("cosT", [128, 2048], f32)
    sinT = din("sinT", [128, 2048], f32)
    mskt = din("mskt", [128, 128], f16)
    iden = din("iden", [128, 128], f16)
    lsel = din("lsel", [2, 1, 128], f16)
    if timing:
        y = nc.dram_tensor("y", [2048, 1024], f16, kind="Internal")
        ytick = nc.declare_dram_parameter("ytick", [128, 128], f16, isOutput=True)
    else:
        y = nc.declare_dram_parameter("y", [2048, 1024], f16, isOutput=True)

    MM = nc.tensor.matmul

    with nc.allow_low_precision(reason="fp16 matmul paths are intentional"), \
         TileContext(nc) as tc:
      for _rep in range(n_reps):
        if _rep:
            tc.strict_bb_all_engine_barrier()
        with _ES() as _loopctx:
          if loop_n:
            _loopctx.enter_context(tc.For_i(0, loop_n, 1))
          with (
            tc.tile_pool(name="persist", bufs=1) as pp,
            tc.tile_pool(name="small", bufs=3) as sp,
          ):
            if timing:
                tk = sp.tile([128, 128], f16, tag="tick")

            nc.gpsimd.load_library(library_config.standard)

            ones_col = pp.tile([128, 1], f16, tag="ones_col")
            nc.gpsimd.memset(ones_col[:], 1.0)
            bias_sb = pp.tile([128, 1], f32, tag="bias")
            nc.gpsimd.memset(bias_sb[:], EXPBIAS)

            qsb = [pp.tile([128, 2048], f16, tag=f"q{h}", name=f"q{h}") for h in range(2)]
            ksb = [pp.tile([128, 2048], f16, tag=f"k{h}", name=f"k{h}") for h in range(2)]
            v_sb = pp.tile([128, 16, 256], f16, tag="v")
            outT = [pp.tile([128, 2048], f16, tag=f"o{h}", name=f"oT{h}") for h in range(2)]
            wo_sb = pp.tile([128, 2, 1024], f16, tag="wo")
            tri_sb = pp.tile([128, 2, 128], f16, tag="tri")
            lsel_sb = [pp.tile([1, 128], f16, tag=f"lsel{i}", name=f"lsel{i}")
                       for i in range(2)]

            # ---------------- Phase 1: QKV + RoPE ----------------
            with (
                tc.tile_pool(name="qkvp", bufs=1) as qp,
                tc.tile_pool(name="rope", bufs=3) as rp,
                tc.tile_pool(name="ps_qkv", bufs=1, space="PSUM") as ps1,
            ):
                x_sb = pp.tile([128, 8, 2048], f16, tag="x")
                wq_sb = qp.tile([128, 2048], f16, tag="wq")
                wk_sb = qp.tile([128, 2048], f16, tag="wk")
                wv_sb = pp.tile([128, 2048], f16, tag="wv")
                cos_sb = qp.tile([128, 2048], f32, tag="cos")
                sin_sb = qp.tile([128, 2048], f32, tag="sin")

                # spread input DMAs across engine queues so they parallelize:
                # x blocks on SP, weights on ACT, cos/sin on DVE, rest on Pool
                nc.scalar.dma_start(out=wq_sb[:], in_=wq[:])
                nc.scalar.dma_start(out=wk_sb[:], in_=wk[:])
                for c4 in range(4):
                    sl = slice(c4 * 512, (c4 + 1) * 512)
                    for et in range(8):
                        nc.sync.dma_start(out=x_sb[:, et, sl], in_=xT[et][:, sl])
                for ch in range(2):
                    csl = slice(ch * 1024, (ch + 1) * 1024)
                    nc.gpsimd.dma_start(out=cos_sb[:, csl], in_=cosT[:, csl])
                    nc.gpsimd.dma_start(out=sin_sb[:, csl], in_=sinT[:, csl])
                nc.gpsimd.dma_start(out=wv_sb[:], in_=wv[:])
                for i in range(2):
                    nc.gpsimd.dma_start(out=tri_sb[:, i, :], in_=mskt[:])
                for i in range(2):
                    nc.gpsimd.dma_start(out=lsel_sb[i][:], in_=lsel[i])
                for hp2 in range(2):
                    nc.gpsimd.dma_start(out=wo_sb[:, hp2, :], in_=wo[hp2])

                # preload the exp table set off the critical path
                warm = sp.tile([128, 1], f32, tag="warm")
                nc.scalar.activation(warm[:], ones_col[:], EXP)

                def qk_rope(hp):
                    for (wsb, dst, tg) in ((wq_sb, qsb, "q"), (wk_sb, ksb, "k")):
                        for c2 in range(2):
                            # [128, 1024] psum pair; rope eviction batched
                            pq = ps1.tile([128, 1024], f32, tag="ps_qk", bufs=3)
                            for half in range(2):
                                sl = slice(c2 * 1024 + half * 512,
                                           c2 * 1024 + (half + 1) * 512)
                                for et in range(8):
                                    MM(pq[:, half * 512:(half + 1) * 512],
                                       wsb[:, (et * 2 + hp) * 128:(et * 2 + hp + 1) * 128],
                                       x_sb[:, et, sl], start=(et == 0), stop=(et == 7))
                            csl = slice(c2 * 1024, (c2 + 1) * 1024)
                            t1 = rp.tile([128, 1024], f32, tag="t1")
                            nc.vector.tensor_mul(t1[:], pq[:], cos_sb[:, csl])
                            rot = rp.tile([128, 1024], f32, tag="rot")
                            nc.vector.stream_shuffle(rot[:], pq[:], PAIRSWAP)
                            t2 = rp.tile([128, 1024], f32, tag="t2")
                            nc.gpsimd.tensor_mul(t2[:], rot[:], sin_sb[:, csl])
                            nc.gpsimd.tensor_add(dst[hp][:, csl], t1[:], t2[:])

                qk_rope(0)
                qk_rope(1)

            # -------- Phase 2+3: attention + outproj, per query superblock --
            with (
                tc.tile_pool(name="probs", bufs=1) as probp,
                tc.tile_pool(name="ps_att", bufs=1, space="PSUM") as ps2,
            ):
                def emit_finalize(I, hp, l_ps, pvp):
                    # 1/l broadcast via two accumulating K=1 selector MMs
                    # (partition-strided APs are rejected by walrus)
                    Isl = slice(I * 512, (I + 1) * 512)
                    rIa = sp.tile([1, 512], f16, tag="rIa")
                    nc.vector.reciprocal(rIa[:], l_ps[0:1, :])
                    rIb = sp.tile([1, 512], f16, tag="rIb")
                    nc.vector.reciprocal(rIb[:], l_ps[64:65, :])
                    Rps = ps2.tile([128, 512], f32, tag="psT", name="Rps",
                                   bufs=3)
                    MM(Rps[:], lsel_sb[0][:], rIa[:], start=True, stop=False)
                    MM(Rps[:], lsel_sb[1][:], rIb[:], start=False, stop=True)
                    Rsb = sp.tile([128, 512], f16, tag="Rsb")
                    nc.vector.tensor_copy(Rsb[:], Rps[:])
                    nc.vector.tensor_mul(outT[hp][:, Isl], pvp[:], Rsb[:])
                    if hp == 1:
                        for t in range(4 * I, 4 * I + 4):
                            for n in range(2):
                                py = ps2.tile([128, 512], f32, tag="psT",
                                              name="py", bufs=3)
                                for h2 in range(2):
                                    MM(py[:],
                                       outT[h2][:, t * 128:(t + 1) * 128],
                                       wo_sb[:, h2, n * 512:(n + 1) * 512],
                                       start=(h2 == 0), stop=(h2 == 1))
                                ysb = sp.tile([128, 512], f16, tag="y_sb")
                                nc.vector.tensor_copy(ysb[:], py[:])
                                nc.sync.dma_start(
                                    out=y[t * 128:(t + 1) * 128,
                                          n * 512:(n + 1) * 512],
                                    in_=ysb[:])

                def v_proj(st0, st1):
                    for st in range(st0, st1):
                        pv = ps2.tile([128, 256], f32, tag="psT", name="pv_ps",
                                      bufs=3)
                        for et in range(8):
                            MM(pv[:], x_sb[:, et, st * 128:(st + 1) * 128],
                               wv_sb[:, et * 256:(et + 1) * 256],
                               start=(et == 0), stop=(et == 7))
                        nc.vector.tensor_copy(v_sb[:, st, :], pv[:])

                vplan = {(0, 0): (0, 4), (0, 1): (4, 8), (1, 1): (8, 12),
                         (2, 0): (12, 16)}
                pending = None
                for I in range(4):
                    for hp in range(2):
                        if (I, hp) in vplan:
                            v_proj(*vplan[(I, hp)])
                        nj = 4 * I + 4
                        l_ps = ps2.tile([128, 512], f32, tag="l", name="l_ps",
                                        bufs=1)
                        pvp = ps2.tile([128, 512], f32, tag="pv", name="pvp",
                                       bufs=1)
                        for j in range(nj):
                            diag = j >= 4 * I
                            m = (j - 4 * I) if diag else 0
                            s = m * 128
                            jj = slice(j * 128, (j + 1) * 128)
                            qsl = slice(I * 512 + s, (I + 1) * 512)
                            psT = ps2.tile([128, 2, 512], f32, tag="psT",
                                           name="psT", bufs=3)
                            for par in range(2):
                                pb = slice(64 * par, 64 * par + 64)
                                MM(psT[:, par, s:], ksb[hp][pb, jj],
                                   qsb[hp][pb, qsl],
                                   start=True, stop=True,
                                   skip_group_check=True)
                            probs = probp.tile([128, 2, 512], f16, tag="probs",
                                               name="probs", bufs=6)
                            if "exp" not in ablate:
                                nc.scalar.activation(probs[:, :, s:],
                                                     psT[:, :, s:],
                                                     EXP, bias=bias_sb[:])
                            else:
                                nc.scalar.activation(probs[:, :, s:s + 64],
                                                     psT[:, :, s:s + 64],
                                                     EXP, bias=bias_sb[:])
                            if diag and "mask" not in ablate:
                                nc.vector.tensor_mul(probs[:, :, s:s + 128],
                                                     probs[:, :, s:s + 128],
                                                     tri_sb[:])
                            if j == 1 and pending is not None:
                                emit_finalize(*pending)
                                pending = None
                            if "l" not in ablate:
                              for par in range(2):
                                MM(l_ps[64 * par:64 * par + 1, s:], ones_col[:],
                                   probs[:, par, s:],
                                   start=(j == 0), stop=(j == nj - 1),
                                   skip_group_check=True,
                                   tile_position=(0, 64 * par))
                            else:
                                MM(l_ps[0:1, s:s + 64], ones_col[:],
                                   probs[:, 0, s:s + 64],
                                   start=(j == 0), stop=(j == nj - 1),
                                   skip_group_check=True)
                            if "pv" not in ablate:
                              for par in range(2):
                                MM(pvp[64 * par:64 * par + 64, s:],
                                   v_sb[:, j, (2 * hp + par) * 64:(2 * hp + par + 1) * 64],
                                   probs[:, par, s:],
                                   start=(j == 0), stop=(j == nj - 1),
                                   skip_group_check=True)
                            else:
                                MM(pvp[0:64, s:s + 64],
                                   v_sb[:, j, (2 * hp) * 64:(2 * hp + 1) * 64],
                                   probs[:, 0, s:s + 64],
                                   start=(j == 0), stop=(j == nj - 1),
                                   skip_group_check=True)
                        pending = (I, hp, l_ps, pvp)
                emit_finalize(*pending)

            if timing:
                nc.sync.dma_start(out=tk[:], in_=y[1920:2048, 896:1024])
                nc.sync.dma_start(out=ytick[:], in_=tk[:])

    nc.finalize()
    return nc


def _host_inputs(x, w_qkv, w_out, core):
    b = core // 4
    g = core % 4
    heads = [4 * g + i for i in range(4)]

    perm = np.empty(64, np.int64)
    for i in range(32):
        perm[2 * i] = i
        perm[2 * i + 1] = i + 32

    xTh = np.ascontiguousarray(x[b].T).astype(np.float16)
    xT = np.empty((8, 128, 2048), np.float16)
    for et in range(8):
        xT[et] = xTh[et * 128:(et + 1) * 128]

    def build_wqk(Wblk, scale):
        out = np.empty((128, 2048), np.float32)
        for et in range(8):
            for hp in range(2):
                cols = np.empty((128, 128), np.float32)
                for par in range(2):
                    h = heads[2 * hp + par]
                    cols[:, par * 64:(par + 1) * 64] = (
                        Wblk[et * 128:(et + 1) * 128, h * 64 + perm])
                out[:, (et * 2 + hp) * 128:(et * 2 + hp + 1) * 128] = cols
        return (out * scale).astype(np.float16)

    wq = build_wqk(w_qkv[:, 0:1024], 0.125)
    wk = build_wqk(w_qkv[:, 1024:2048], 1.0)

    wv = np.empty((128, 2048), np.float16)
    for et in range(8):
        for hl in range(4):
            wv[:, et * 256 + hl * 64: et * 256 + (hl + 1) * 64] = (
                w_qkv[et * 128:(et + 1) * 128,
                      2048 + heads[hl] * 64: 2048 + (heads[hl] + 1) * 64])

    wo = np.empty((2, 128, 1024), np.float16)
    for hp in range(2):
        for par in range(2):
            h = heads[2 * hp + par]
            wo[hp, par * 64:(par + 1) * 64, :] = (
                w_out[h * 64:(h + 1) * 64, :].astype(np.float16))

    invf = 1.0 / (THETA ** (np.arange(0, D, 2, dtype=np.float64) / D))
    ang = np.arange(S, dtype=np.float64)[:, None] * invf[None, :]
    cosv = np.cos(ang).astype(np.float32)
    sinv = np.sin(ang).astype(np.float32)
    cosT = np.empty((128, 2048), np.float32)
    sinT = np.empty((128, 2048), np.float32)
    for r in range(64):
        i = r // 2
        sgn = -1.0 if (r % 2 == 0) else 1.0
        cosT[r, :] = cosv[:, i]
        cosT[r + 64, :] = cosv[:, i]
        sinT[r, :] = sgn * sinv[:, i]
        sinT[r + 64, :] = sgn * sinv[:, i]

    # [128,128] lower-triangle mask: mskt[p, c] = 0 if c >= p else -30000
    p_ = np.arange(128)[:, None]
    c_ = np.arange(128)[None, :]
    mskt = np.where(c_ >= p_, 1.0, 0.0).astype(np.float16)
    iden = np.eye(128, dtype=np.float16)
    lsel = np.zeros((2, 1, 128), np.float16)
    lsel[0, 0, 0:64] = 1.0
    lsel[1, 0, 64:128] = 1.0

    return {
        "xT": xT, "wq": wq, "wk": wk, "wv": wv, "wo": wo,
        "cosT": cosT, "sinT": sinT, "mskt": mskt, "iden": iden, "lsel": lsel,
    }


def kernel(x, w_qkv, w_out, trace=False):
    from concourse.bass_utils import run_bass_kernel_spmd

    x = np.asarray(x, np.float32)
    w_qkv = np.asarray(w_qkv, np.float32)
    w_out = np.asarray(w_out, np.float32)

    if "nc" not in _PROG_CACHE:
        _PROG_CACHE["nc"] = _build_program()
    nc = _PROG_CACHE["nc"]

    in_maps = [_host_inputs(x, w_qkv, w_out, c) for c in range(8)]
    res = run_bass_kernel_spmd(nc, in_maps, list(range(8)), trace=trace)
    _PROG_CACHE["last_result"] = res

    y0 = sum(res.results[c]["y"].astype(np.float64) for c in range(4))
    y1 = sum(res.results[c]["y"].astype(np.float64) for c in range(4, 8))
    return np.stack([y0, y1]).astype(np.float32)
